# revision 42
# baseline (speedup 1.0000x reference)
"""Balanced BCE loss kernel for Trainium2 (8 NeuronCores, SPMD).

Math: the loss needs, per class c, the sums
    S_all[c] = sum_b softplus(x),  S1[c] = sum_b t * softplus(x)
with x = (1-2t)*pred and softplus(x) = -ln sigmoid(-x). Both sums are
order-invariant over the batch, so the HOST counting-sorts each class's
65536 elements t-first and deals them round-robin to the 8 cores. Per
(core, class) row of 8192 sorted columns:
  - cols [0, 1792)      : guaranteed pure t=1  -> device slab 0
  - cols [1792, 2816)   : boundary window      -> HOST (1024 cols, f64;
                          the t=1/t=0 boundary p1 = #(t=1)/8 ~ 2457 +- 15
                          for Bernoulli(0.3) targets: +-20 sigma margins)
  - cols [2816, 8192)   : guaranteed pure t=0  -> device slabs 1-4
Device rows are 7168 fp8 columns in slabs of (1792 | 1792,1792,1280,384,128),
transposed [class, col] (class = partition, 4 blocks of 128 classes,
processed as two 2-block superblocks). The ACT sigmoid chain
s = Sigmoid(-x) (fp8 -> bf16, 1 elem/lane/cycle @1.2GHz, ~26us/core) is the
critical path; everything else is scheduled to hide inside it:
  - the first 1792 cols of sb0 live in a single-buffered prefetch tile that
    iteration i-1 reloads for iteration i, so the first sigmoid issues at
    the top of the loop body instead of waiting out the post-barrier DMA
    latency (~2.4us); the remaining x streams on SWDGE in slab-sized chunks
  - per-slab DVE halving trees collapse each t-pure slab into groups of 8:
    ln prod = sum ln s, so the host recovers exact per-group log-sigmoid
    sums with one np.log per product (group purity holds: each group lives
    inside one t-pure slab of one class); tapered slab widths let the tree
    of slab k hide under the ACT of slab k+1
  - the three 1792-slabs use k=8 trees; the late 1280/384 slabs use
    shallower k=4 / k=2 trees and the last 128-col ACT piece writes raw
    bf16 sigmoids, all landing in a persistent tail tile, so almost no DVE
    work remains after the final sigmoid
  - products ship as one bulk DMA per superblock (first 3 tree sets,
    issued mid-chain) plus ONE deferred tail DMA issued at the start of
    the NEXT iteration — its issue and ~1.7us queue drain hide under the
    ACT chain instead of extending the end-of-iteration barrier
    (single-shot NEFFs ship the tail at the end instead). 1.29 MiB/core
    out vs 8 MiB/core for the previous pair-product kernel; total HBM
    traffic 4.8 MiB/core vs 12.
Host finalize: group logs + window softplus + exact integer bookkeeping of
the t=1 prefix per (core, class); any row whose boundary leaves the window
(impossible for the reference distribution) is recomputed exactly on host
from the same fp8 values the device saw.

Measured (loop-differenced through run_bass_kernel_spmd, 8 cores, same day
and method): ~31us/iter vs 53.4us/iter for the previous pair-product
kernel. CoreSim models 26.7us/iter steady (ACT chain 25.9us + ~0.8us
barrier) vs 35.0us; the larger real-path gain comes from shedding the old
kernel's HBM contention (12 MiB/core).
"""

import sys
import time
from contextlib import ExitStack

import numpy as np
import ml_dtypes

sys.path.insert(0, "/opt/trn_rl_repo")

from concourse import bacc, mybir, tile  # noqa: E402
from concourse.bass_utils import run_bass_kernel_spmd  # noqa: E402

B, C = 65536, 512
N_CORES = 8
P = 128
NBLK = C // P            # 4 class blocks of 128
NSB = 2                  # superblocks (2 blocks each)
COLS = B // N_CORES      # 8192 sorted columns per (core, class)
T1 = 1792                # t=1-pure device cols (slab 0)
WHOST = 1024             # host window cols, sorted order [T1, T1+WHOST)
DCOLS = COLS - WHOST     # 7168 device cols per class row
T0 = DCOLS - T1          # 5376 t=0-pure device cols
K_GROUP = 8              # sigmoids per product group (3 halving levels)

# tree sets: (col offset, n equal slabs, slab width, group size); first set
# is the t1 slab; the late sets use shallower trees (k=4 / k=2) so their
# final DVE level lands right behind the last sigmoid
TREES = ((0, 1, 1792, 8), (1792, 1, 1792, 8), (3584, 1, 1792, 8),
         (5376, 1, 1280, 4), (6656, 1, 384, 2))
RAW_OFF = 7040           # last 128 cols ship as raw bf16 sigmoids: their
RAW_W = DCOLS - RAW_OFF  # ACT piece writes straight into the gather tile,
                         # so no DVE work remains after the final sigmoid
# ACT piece column boundaries per superblock (tapered tail: the tree of
# piece k hides under the ACT of piece k+1). sb0's t1+t0 head is one piece
# (prefetched, data ready at body start); sb1 keeps finer pieces so its
# trees start early enough to hide under the remaining ACT work.
ACT_BOUNDS_SB = (
    (0, 1792, 5376, 7040, 7168),
    (0, 1792, 3584, 5376, 6656, 7040, 7168),
)
G_PROD = sum(ns * w // k for _, ns, w, k in TREES)       # 1184 products per row
GROW = G_PROD + RAW_W                                    # 1312 out cols per row
BULK = sum(ns * w // k for _, ns, w, k in TREES[:3])     # 672: early-out sets
TAILW = GROW - BULK                                      # 640: deferred tail

F32 = mybir.dt.float32
BF16 = mybir.dt.bfloat16
FP8 = mybir.dt.float8e4

FP8_NP = ml_dtypes.float8_e4m3
BF16_NP = ml_dtypes.bfloat16

_CACHE = {}


def _build(loop_n: int = 1, io_bufs: int = 2, s_bufs: int = 2, w_bufs: int = 2,
           mode: str = "full", last_out_scalar: bool = False,
           dma_bounds: tuple = (0, 448, 1792, 3584, 5376, 7168),
           n_hwdge_chunks: int = 0, out_split: int = 3, prefetch: int = 1792):
    nc = bacc.Bacc(
        "TRN2", target_bir_lowering=False, debug=False, num_devices=N_CORES
    )
    x = nc.dram_tensor("x", [C, DCOLS], FP8, kind="ExternalInput").ap()
    prod = nc.dram_tensor(
        "prod", [P, NSB * 2 * GROW], BF16, kind="ExternalOutput"
    ).ap()
    prod_v = prod.rearrange("p (s b g) -> p s b g", s=NSB, b=2)
    # partition p of superblock sb holds classes sb*256 + {p, 128+p}
    x_v = x.rearrange("(sb b p) q -> sb p b q", p=P, b=2)

    with tile.TileContext(nc) as tc, ExitStack() as stack:
        io = stack.enter_context(tc.tile_pool(name="io", bufs=io_bufs))
        sp = stack.enter_context(tc.tile_pool(name="sp", bufs=s_bufs))
        wk = stack.enter_context(tc.tile_pool(name="wk", bufs=w_bufs))
        pf = stack.enter_context(tc.tile_pool(name="pf", bufs=1))
        # hoist the sigmoid ACT_TABLE_LOAD out of the loop body (the DMA of
        # the result keeps the warm activation from being dead-code removed)
        warm = wk.tile([P, 2], BF16, tag="warm")
        nc.vector.memset(warm[:], 0.0)
        nc.scalar.activation(
            warm[:], warm[:], mybir.ActivationFunctionType.Sigmoid, scale=-1.0
        )
        nc.sync.dma_start(out=prod[:, 0:2], in_=warm[:])
        # software pipeline: sb0's first `prefetch` columns live in a
        # single-buffered tile that iteration i-1 loads for iteration i, so
        # the first sigmoid starts right at the top of the body instead of
        # waiting out the post-barrier DMA latency
        if prefetch:
            x_pf = pf.tile([P, 2, prefetch], FP8, tag="xpf")
            nc.gpsimd.dma_start(out=x_pf[:], in_=x_v[0, :, :, 0:prefetch])
        # tail products live in a single-buffered tile whose DMA-out is
        # deferred to the START of the next iteration, so its issue and
        # queue-drain hide under the ACT chain instead of extending the
        # end-of-iteration barrier (iteration 0 ships the memset content)
        tail_t = pf.tile([P, NSB, 2, TAILW], BF16, tag="tail")
        nc.vector.memset(tail_t[:], 1.0)
        if loop_n > 1:
            stack.enter_context(tc.For_i(0, loop_n, 1))
            if mode != "act":
                nc.sync.dma_start(
                    out=prod_v[:, :, :, BULK:GROW], in_=tail_t[:]
                )

        n_tree_sets = len(TREES)
        for sb in range(NSB):
            x_t = io.tile([P, 2, DCOLS], FP8, tag="x")
            pf_here = prefetch if sb == 0 else 0
            for ci, (a, b) in enumerate(zip(dma_bounds[:-1], dma_bounds[1:])):
                if b <= pf_here:
                    continue  # covered by the prefetch tile
                a = max(a, pf_here)
                eng = nc.sync if (sb == 0 and ci < n_hwdge_chunks) else nc.gpsimd
                eng.dma_start(out=x_t[:, :, a:b], in_=x_v[sb, :, :, a:b])
            s_t = sp.tile([P, 2, DCOLS], BF16, tag="s")
            gather = wk.tile([P, 2, BULK], BF16, tag="gather")
            done = 0
            ti = 0  # next tree set awaiting activation coverage
            goff = 0
            act_bounds = ACT_BOUNDS_SB[sb]
            for a, b in zip(act_bounds[:-1], act_bounds[1:]):
                if b <= pf_here:
                    src_x = x_pf[:, :, a:b]
                else:
                    src_x = x_t[:, :, a:b]
                if a >= RAW_OFF:
                    # raw-sigmoid tail goes straight into the tail tile
                    o0 = G_PROD - BULK + a - RAW_OFF
                    dst = tail_t[:, sb, :, o0:o0 + b - a]
                else:
                    dst = s_t[:, :, a:b]
                nc.scalar.activation(
                    dst, src_x,
                    mybir.ActivationFunctionType.Sigmoid, scale=-1.0,
                )
                done = b
                if pf_here and done == pf_here:
                    # refill the prefetch tile for the next iteration now
                    # that its last reader has been issued
                    nc.gpsimd.dma_start(
                        out=x_pf[:], in_=x_v[0, :, :, 0:prefetch]
                    )
                if mode == "act":
                    continue
                # emit trees whose slabs are fully activated; the final level
                # of each tree writes into the contiguous gather tile so one
                # DMA per superblock ships all products
                while ti < n_tree_sets:
                    off, nsl, w, kg = TREES[ti]
                    if off + nsl * w > done:
                        break
                    src = s_t[:, :, off:off + nsl * w].rearrange(
                        "p b (v q) -> p b v q", v=nsl
                    )
                    cur = w
                    g = w // kg
                    ng = nsl * g
                    while cur > g:
                        half = cur // 2
                        if half == g and goff + ng <= BULK:
                            nxt = gather[:, :, goff:goff + ng].rearrange(
                                "p b (v q) -> p b v q", v=nsl
                            )
                        elif half == g:
                            to = goff - BULK
                            nxt = tail_t[:, sb, :, to:to + ng].rearrange(
                                "p b (v q) -> p b v q", v=nsl
                            )
                        else:
                            nxt = wk.tile(
                                [P, 2, nsl, half], BF16, tag=f"t{off}_{half}"
                            )
                        nc.vector.tensor_mul(
                            nxt[:], src[:, :, :, 0:half], src[:, :, :, half:cur]
                        )
                        src, cur = nxt, half
                    goff += ng
                    ti += 1
                    # ship the bulk of the products as soon as the first
                    # three tree sets land; the rest rides the deferred tail
                    if mode == "full" and goff == BULK:
                        nc.sync.dma_start(
                            out=prod_v[:, sb, :, 0:BULK],
                            in_=gather[:, :, 0:BULK],
                        )
            if mode == "full" and loop_n == 1 and sb == NSB - 1:
                # single-shot: no next iteration to ship the tail
                nc.sync.dma_start(
                    out=prod_v[:, :, :, BULK:GROW], in_=tail_t[:]
                )
        if mode != "full":
            dummy = wk.tile([P, 2], BF16, tag="dummy")
            nc.vector.memset(dummy[:], 1.0)
            nc.sync.dma_start(out=prod[:, 0:2], in_=dummy[:])

    nc.compile()
    return nc


def _get_nc(loop_n: int = 1, **kw):
    key = (loop_n, tuple(sorted(kw.items())))
    if key not in _CACHE:
        _CACHE[key] = _build(loop_n, **kw)
    return _CACHE[key]


DEFAULT_KW = dict()


def _prep_inputs(pred: np.ndarray, target: np.ndarray):
    """Host-side: per-class counting sort by t, deal to cores, split into
    device (t-pure) columns and host (boundary window) columns.

    Returns (xs, xw, n1):
      xs: [N_CORES, C, DCOLS] fp8 device inputs (slab0 t1 | slabs1-4 t0)
      xw: [N_CORES, C, WHOST] f32 host window columns
      n1: [C] int64 per-class t=1 counts
    """
    t = target != 0.0
    n1 = t.sum(axis=0, dtype=np.int64)
    x = ((1.0 - 2.0 * target) * pred).astype(FP8_NP)

    # global sorted position per element: t=1 elements first (stable order);
    # rank among t=0 = row_index - (#t=1 so far), so one cumsum suffices
    c1 = np.cumsum(t, axis=0, dtype=np.int32)
    rows = np.arange(B, dtype=np.int32)[:, None]
    pos = np.where(t, c1 - 1, n1.astype(np.int32)[None, :] + rows - c1)
    # deal to cores round-robin; scatter everything into a staging array
    # [core, class, col] and slice device/window parts out afterwards
    cc = np.broadcast_to(np.arange(C, dtype=np.int32)[None, :], pos.shape)
    lin = ((pos & 7) * C + cc) * COLS + (pos >> 3)
    stage = np.empty(N_CORES * C * COLS, dtype=np.uint8)
    stage[lin.reshape(-1)] = x.view(np.uint8).reshape(-1)
    stage = stage.reshape(N_CORES, C, COLS)

    xs = np.empty((N_CORES, C, DCOLS), dtype=np.uint8)
    xs[:, :, :T1] = stage[:, :, :T1]
    xs[:, :, T1:] = stage[:, :, T1 + WHOST:]
    xw = stage[:, :, T1:T1 + WHOST]

    xs = xs.view(FP8_NP)
    xw_f = xw.view(FP8_NP).astype(np.float32)
    return xs, xw_f, n1


def run_device(xs: np.ndarray, loop_n: int = 1, **kw):
    nc = _get_nc(loop_n, **{**DEFAULT_KW, **kw})
    in_maps = [{"x": np.ascontiguousarray(xs[i])} for i in range(N_CORES)]
    results = None
    for attempt in range(3):
        try:
            results = run_bass_kernel_spmd(nc, in_maps, list(range(N_CORES))).results
            break
        except Exception:
            if attempt == 2:
                raise
            time.sleep(5)
            try:
                import jax
                import jax.extend.backend as _jax_backend

                jax.clear_caches()
                _jax_backend.clear_backends()
            except Exception:
                pass
    return [r["prod"] for r in results]


def _softplus64(x):
    return np.logaddexp(0.0, x)


def _reconstruct(prods, xw, n1):
    """prods: per-core [P, NSB*2*GROW] bf16 slab-group sigmoid products.
    Returns per-(core, class) partial sums of softplus / t*softplus."""
    i_idx = np.arange(N_CORES, dtype=np.int64)[:, None]
    # p1[i, c] = #(t=1) dealt to core i = #{j < n1[c] : j % 8 == i}
    p1 = np.maximum(n1[None, :] - i_idx + 7, 0) // 8          # [8, C]

    g1 = TREES[0][2] // TREES[0][3]      # groups in the t1 slab (first set)
    dev_all = np.zeros((N_CORES, C), dtype=np.float64)
    dev_t1 = np.zeros((N_CORES, C), dtype=np.float64)
    for i, o in enumerate(prods):
        v = np.asarray(o).astype(np.float32).astype(np.float64)
        v = v.reshape(P, NSB, 2, GROW)              # (p, sb, b, g)
        lg = np.log(v)
        # class c = (sb*2 + b)*128 + p
        t1_sum = -lg[:, :, :, :g1].sum(axis=3)      # (p, sb, b)
        t0_sum = -lg[:, :, :, g1:].sum(axis=3)
        dev_t1[i] = t1_sum.transpose(1, 2, 0).reshape(C)
        dev_all[i] = dev_t1[i] + t0_sum.transpose(1, 2, 0).reshape(C)

    # host window: softplus of xw in f64; prefix sums for the t-boundary
    spw = _softplus64(xw.astype(np.float64))            # [8, C, WHOST]
    win_all = spw.sum(axis=2)                           # [8, C]
    cums = np.concatenate(
        [np.zeros((N_CORES, C, 1)), np.cumsum(spw, axis=2)], axis=2
    )
    wlen = np.clip(p1 - T1, 0, WHOST)
    win_t1 = np.take_along_axis(cums, wlen[:, :, None], axis=2)[:, :, 0]

    part_all = dev_all + win_all
    part_t1 = dev_t1 + win_t1

    bad = (p1 < T1) | (p1 > T1 + WHOST)
    return part_all, part_t1, bad, p1


def _finalize(s_all, s1, pos_sum, pos_prop) -> np.ndarray:
    bal = pos_prop.astype(np.float64) * B
    maj1 = pos_sum >= bal
    n_maj = np.where(maj1, pos_sum, B - pos_sum)
    n_min = B - n_maj
    s_maj = np.where(maj1, s1, s_all - s1)
    s_min = s_all - s_maj
    w_maj = bal / np.maximum(n_maj, 1.0)
    w_min = np.where(n_min > 0, (B - bal) / np.maximum(n_min, 1.0), 1.0)
    loss = (np.where(s_maj == 0, 0.0, w_maj * s_maj) + w_min * s_min).sum() / (B * C)
    return np.asarray(loss, dtype=np.float32)


def kernel(pred: np.ndarray, target: np.ndarray, pos_prop: np.ndarray) -> np.ndarray:
    pred = np.asarray(pred, dtype=np.float32)
    target = np.asarray(target, dtype=np.float32)
    pos_prop = np.asarray(pos_prop, dtype=np.float32)
    pos_sum = target.astype(np.float64).sum(axis=0)

    xs, xw, n1 = _prep_inputs(pred, target)
    prods = run_device(xs)
    part_all, part_t1, bad, p1 = _reconstruct(prods, xw, n1)

    if bad.any():
        # exact host recompute from the same fp8 values the device saw
        for i, c in zip(*np.nonzero(bad)):
            sp_dev = _softplus64(xs[i, c].astype(np.float64))
            spw = _softplus64(xw[i, c].astype(np.float64))
            part_all[i, c] = sp_dev.sum() + spw.sum()
            k = int(p1[i, c])
            if k <= T1:
                part_t1[i, c] = sp_dev[:k].sum()
            elif k <= T1 + WHOST:
                part_t1[i, c] = sp_dev[:T1].sum() + spw[: k - T1].sum()
            else:
                part_t1[i, c] = sp_dev[: k - WHOST].sum() + spw.sum()

    s_all = part_all.sum(axis=0)
    s1 = part_t1.sum(axis=0)
    return _finalize(s_all, s1, pos_sum, pos_prop)


# ---------------- benchmarking -----------------------------------------------


def bench_spmd(pred: np.ndarray, target: np.ndarray, loop_small: int = 101,
               loop_big: int = 1101, reps: int = 5, **kw):
    """Per-iteration device time via For_i loop differencing through the
    run_bass_kernel_spmd path (all 8 cores concurrently)."""
    xs, _, _ = _prep_inputs(
        np.asarray(pred, dtype=np.float32), np.asarray(target, dtype=np.float32)
    )
    ts, tb = [], []
    run_device(xs, loop_n=loop_small, **kw)  # compile+warm
    run_device(xs, loop_n=loop_big, **kw)
    for _ in range(reps):
        t0 = time.perf_counter()
        run_device(xs, loop_n=loop_small, **kw)
        ts.append(time.perf_counter() - t0)
        t0 = time.perf_counter()
        run_device(xs, loop_n=loop_big, **kw)
        tb.append(time.perf_counter() - t0)
    ns = (min(tb) - min(ts)) / (loop_big - loop_small) * 1e9
    return ns, min(ts), min(tb)


if __name__ == "__main__":
    rng = np.random.default_rng(0)
    pred = rng.standard_normal((B, C), dtype=np.float32)
    target = (rng.random((B, C)) < 0.3).astype(np.float32)
    pos_prop = np.full((C,), 0.5, dtype=np.float32)
    print(kernel(pred, target, pos_prop))


# revision 47
# speedup vs baseline: 1.0650x; 1.0650x over previous
"""Balanced BCE loss kernel for Trainium2 (8 NeuronCores, SPMD).

Math: the loss needs, per class c, the sums
    S_all[c] = sum_b softplus(x),  S1[c] = sum_b t * softplus(x)
with x = (1-2t)*pred and softplus(x) = -ln sigmoid(-x). Both sums are
order-invariant over the batch, so the HOST counting-sorts each class's
65536 elements t-first and deals them round-robin to the 8 cores. Per
(core, class) row of 8192 sorted columns:
  - cols [0, 1792)      : guaranteed pure t=1  -> device slab 0
  - cols [1792, 2816)   : boundary window      -> HOST (1024 cols, f64;
                          the t=1/t=0 boundary p1 = #(t=1)/8 ~ 2457 +- 15
                          for Bernoulli(0.3) targets: +-20 sigma margins)
  - cols [2816, 8192)   : guaranteed pure t=0  -> device slabs 1-4
Device rows are 7168 fp8 columns in slabs of (1792 | 1792,1792,1280,384,128),
transposed [class, col]: one [128, 4, 7168] block per iteration (partition p
holds classes {p, 128+p, 256+p, 384+p}). The ACT sigmoid chain
s = Sigmoid(-x) (fp8 -> bf16, 1 elem/lane/cycle @1.2GHz, 23.9us/core of
elements + 6 x ~185ns instruction overheads) is the critical path;
everything else is scheduled to hide inside it:
  - the first 1792 cols live in a single-buffered prefetch tile that
    iteration i-1 reloads for iteration i, so the first sigmoid issues at
    the top of the loop body instead of waiting out the post-barrier DMA
    latency (~2.4us); the remaining x streams on SWDGE in slab-sized chunks
  - per-slab DVE halving trees collapse each t-pure slab into groups:
    ln prod = sum ln s, so the host recovers exact per-group log-sigmoid
    sums with one np.log per product (group purity holds: each group lives
    inside one t-pure slab of one class); tapered slab widths let the tree
    of slab k hide under the ACT of slab k+1
  - the three 1792-slabs use k=8 trees; the late 1280/384 slabs use
    shallow k=2 trees and the last 128-col ACT piece writes raw bf16
    sigmoids, all landing in a persistent tail tile, so almost no DVE work
    remains after the final sigmoid
  - products ship as one bulk DMA (first 3 tree sets, issued mid-chain)
    plus ONE deferred tail DMA issued at the start of the NEXT iteration -
    its issue and ~1.7us queue drain hide under the ACT chain instead of
    extending the end-of-iteration barrier (single-shot NEFFs ship the
    tail at the end instead). 1.6 MiB/core out vs 8 MiB/core for the
    previous pair-product kernel; total HBM traffic 5.1 MiB/core vs 12.
Host finalize: group logs + window softplus + exact integer bookkeeping of
the t=1 prefix per (core, class); any row whose boundary leaves the window
(impossible for the reference distribution) is recomputed exactly on host
from the same fp8 values the device saw.

Measured (loop-differenced through run_bass_kernel_spmd, 8 cores, same day
and method): ~29-31us/iter vs 53.4us/iter for the previous pair-product
kernel under +-3us tunnel noise. CoreSim models 25.7us/iter steady (ACT
chain 25.0us + ~0.7us barrier) vs 35.0us for the old kernel; the larger
real-path gain comes from shedding the old kernel's HBM contention.
"""

import sys
import time
from contextlib import ExitStack

import numpy as np
import ml_dtypes

sys.path.insert(0, "/opt/trn_rl_repo")

from concourse import bacc, mybir, tile  # noqa: E402
from concourse.bass_utils import run_bass_kernel_spmd  # noqa: E402

B, C = 65536, 512
N_CORES = 8
P = 128
ROWS = C // P            # 4 class rows of 128 per partition, one block
COLS = B // N_CORES      # 8192 sorted columns per (core, class)
T1 = 1792                # t=1-pure device cols (slab 0)
WHOST = 1024             # host window cols, sorted order [T1, T1+WHOST)
DCOLS = COLS - WHOST     # 7168 device cols per class row
T0 = DCOLS - T1          # 5376 t=0-pure device cols
K_GROUP = 8              # sigmoids per product group (3 halving levels)

# tree sets: (col offset, n equal slabs, slab width, group size); first set
# is the t1 slab; the late sets use shallower trees (k=2) so their
# final DVE level lands right behind the last sigmoid
TREES = ((0, 1, 1792, 8), (1792, 1, 1792, 8), (3584, 1, 1792, 8),
         (5376, 1, 1280, 2), (6656, 1, 384, 2))
RAW_OFF = 7040           # last 128 cols ship as raw bf16 sigmoids: their
RAW_W = DCOLS - RAW_OFF  # ACT piece writes straight into the tail tile,
                         # so no DVE work remains after the final sigmoid
# ACT piece column boundaries (tapered tail: the tree of piece k hides
# under the ACT of piece k+1; the [0:1792) head is prefetched so its data
# is ready at body start)
ACT_BOUNDS = (0, 1792, 3584, 5376, 6656, 7040, 7168)
G_PROD = sum(ns * w // k for _, ns, w, k in TREES)       # 1504 products per row
GROW = G_PROD + RAW_W                                    # 1632 out cols per row
BULK = sum(ns * w // k for _, ns, w, k in TREES[:3])     # 672: early-out sets
TAILW = GROW - BULK                                      # 960: deferred tail

F32 = mybir.dt.float32
BF16 = mybir.dt.bfloat16
FP8 = mybir.dt.float8e4

FP8_NP = ml_dtypes.float8_e4m3
BF16_NP = ml_dtypes.bfloat16

_CACHE = {}


def _build(loop_n: int = 1, w_bufs: int = 2, mode: str = "full",
           dma_bounds: tuple = (0, 1792, 3584, 5376, 7168),
           prefetch: int = 1792):
    nc = bacc.Bacc(
        "TRN2", target_bir_lowering=False, debug=False, num_devices=N_CORES
    )
    x = nc.dram_tensor("x", [C, DCOLS], FP8, kind="ExternalInput").ap()
    prod = nc.dram_tensor(
        "prod", [P, ROWS * GROW], BF16, kind="ExternalOutput"
    ).ap()
    prod_v = prod.rearrange("p (b g) -> p b g", b=ROWS)
    # partition p holds classes {p, 128+p, 256+p, 384+p} (row b = class block)
    x_v = x.rearrange("(b p) q -> p b q", p=P)

    with tile.TileContext(nc) as tc, ExitStack() as stack:
        io = stack.enter_context(tc.tile_pool(name="io", bufs=1))
        sp = stack.enter_context(tc.tile_pool(name="sp", bufs=1))
        wk = stack.enter_context(tc.tile_pool(name="wk", bufs=w_bufs))
        pf = stack.enter_context(tc.tile_pool(name="pf", bufs=1))
        # hoist the sigmoid ACT_TABLE_LOAD out of the loop body (the DMA of
        # the result keeps the warm activation from being dead-code removed)
        warm = wk.tile([P, 2], BF16, tag="warm")
        nc.vector.memset(warm[:], 0.0)
        nc.scalar.activation(
            warm[:], warm[:], mybir.ActivationFunctionType.Sigmoid, scale=-1.0
        )
        nc.sync.dma_start(out=prod[:, 0:2], in_=warm[:])
        # software pipeline: the first `prefetch` columns live in a
        # single-buffered tile that iteration i-1 loads for iteration i, so
        # the first sigmoid starts right at the top of the body instead of
        # waiting out the post-barrier DMA latency
        if prefetch:
            x_pf = pf.tile([P, ROWS, prefetch], FP8, tag="xpf")
            nc.gpsimd.dma_start(out=x_pf[:], in_=x_v[:, :, 0:prefetch])
        # tail products live in a single-buffered tile whose DMA-out is
        # deferred to the START of the next iteration, so its issue and
        # queue-drain hide under the ACT chain instead of extending the
        # end-of-iteration barrier (iteration 0 ships the memset content)
        tail_t = pf.tile([P, ROWS, TAILW], BF16, tag="tail")
        nc.vector.memset(tail_t[:], 1.0)
        if loop_n > 1:
            stack.enter_context(tc.For_i(0, loop_n, 1))
            if mode != "act":
                nc.sync.dma_start(
                    out=prod_v[:, :, BULK:GROW], in_=tail_t[:]
                )

        n_tree_sets = len(TREES)
        x_t = io.tile([P, ROWS, DCOLS], FP8, tag="x")
        for a, b in zip(dma_bounds[:-1], dma_bounds[1:]):
            if b <= prefetch:
                continue  # covered by the prefetch tile
            a = max(a, prefetch)
            nc.gpsimd.dma_start(out=x_t[:, :, a:b], in_=x_v[:, :, a:b])
        s_t = sp.tile([P, ROWS, DCOLS], BF16, tag="s")
        gather = wk.tile([P, ROWS, BULK], BF16, tag="gather")
        done = 0
        ti = 0  # next tree set awaiting activation coverage
        goff = 0
        for a, b in zip(ACT_BOUNDS[:-1], ACT_BOUNDS[1:]):
            if b <= prefetch:
                src_x = x_pf[:, :, a:b]
            else:
                src_x = x_t[:, :, a:b]
            if a >= RAW_OFF:
                # raw-sigmoid tail goes straight into the tail tile
                o0 = G_PROD - BULK + a - RAW_OFF
                dst = tail_t[:, :, o0:o0 + b - a]
            else:
                dst = s_t[:, :, a:b]
            nc.scalar.activation(
                dst, src_x,
                mybir.ActivationFunctionType.Sigmoid, scale=-1.0,
            )
            done = b
            if prefetch and done == prefetch:
                # refill the prefetch tile for the next iteration now
                # that its last reader has been issued
                nc.gpsimd.dma_start(
                    out=x_pf[:], in_=x_v[:, :, 0:prefetch]
                )
            if mode == "act":
                continue
            # emit trees whose slabs are fully activated; final levels write
            # into the contiguous gather / tail tiles
            while ti < n_tree_sets:
                off, nsl, w, kg = TREES[ti]
                if off + nsl * w > done:
                    break
                src = s_t[:, :, off:off + nsl * w].rearrange(
                    "p b (v q) -> p b v q", v=nsl
                )
                cur = w
                g = w // kg
                ng = nsl * g
                while cur > g:
                    half = cur // 2
                    if half == g and goff + ng <= BULK:
                        nxt = gather[:, :, goff:goff + ng].rearrange(
                            "p b (v q) -> p b v q", v=nsl
                        )
                    elif half == g:
                        to = goff - BULK
                        nxt = tail_t[:, :, to:to + ng].rearrange(
                            "p b (v q) -> p b v q", v=nsl
                        )
                    else:
                        nxt = wk.tile(
                            [P, ROWS, nsl, half], BF16, tag=f"t{off}_{half}"
                        )
                    nc.vector.tensor_mul(
                        nxt[:], src[:, :, :, 0:half], src[:, :, :, half:cur]
                    )
                    src, cur = nxt, half
                goff += ng
                ti += 1
                # ship the bulk of the products as soon as the first
                # three tree sets land; the rest rides the deferred tail
                if mode == "full" and goff == BULK:
                    nc.sync.dma_start(
                        out=prod_v[:, :, 0:BULK],
                        in_=gather[:, :, 0:BULK],
                    )
        if mode == "full" and loop_n == 1:
            # single-shot: no next iteration to ship the tail
            nc.sync.dma_start(
                out=prod_v[:, :, BULK:GROW], in_=tail_t[:]
            )
        if mode != "full":
            dummy = wk.tile([P, 2], BF16, tag="dummy")
            nc.vector.memset(dummy[:], 1.0)
            nc.sync.dma_start(out=prod[:, 0:2], in_=dummy[:])

    nc.compile()
    return nc


def _get_nc(loop_n: int = 1, **kw):
    key = (loop_n, tuple(sorted(kw.items())))
    if key not in _CACHE:
        _CACHE[key] = _build(loop_n, **kw)
    return _CACHE[key]


DEFAULT_KW = dict()


def _prep_inputs(pred: np.ndarray, target: np.ndarray):
    """Host-side: per-class counting sort by t, deal to cores, split into
    device (t-pure) columns and host (boundary window) columns.

    Returns (xs, xw, n1):
      xs: [N_CORES, C, DCOLS] fp8 device inputs (slab0 t1 | slabs1-4 t0)
      xw: [N_CORES, C, WHOST] f32 host window columns
      n1: [C] int64 per-class t=1 counts
    """
    t = target != 0.0
    n1 = t.sum(axis=0, dtype=np.int64)
    x = ((1.0 - 2.0 * target) * pred).astype(FP8_NP)

    # global sorted position per element: t=1 elements first (stable order);
    # rank among t=0 = row_index - (#t=1 so far), so one cumsum suffices
    c1 = np.cumsum(t, axis=0, dtype=np.int32)
    rows = np.arange(B, dtype=np.int32)[:, None]
    pos = np.where(t, c1 - 1, n1.astype(np.int32)[None, :] + rows - c1)
    # deal to cores round-robin; scatter everything into a staging array
    # [core, class, col] and slice device/window parts out afterwards
    cc = np.broadcast_to(np.arange(C, dtype=np.int32)[None, :], pos.shape)
    lin = ((pos & 7) * C + cc) * COLS + (pos >> 3)
    stage = np.empty(N_CORES * C * COLS, dtype=np.uint8)
    stage[lin.reshape(-1)] = x.view(np.uint8).reshape(-1)
    stage = stage.reshape(N_CORES, C, COLS)

    xs = np.empty((N_CORES, C, DCOLS), dtype=np.uint8)
    xs[:, :, :T1] = stage[:, :, :T1]
    xs[:, :, T1:] = stage[:, :, T1 + WHOST:]
    xw = stage[:, :, T1:T1 + WHOST]

    xs = xs.view(FP8_NP)
    xw_f = xw.view(FP8_NP).astype(np.float32)
    return xs, xw_f, n1


def run_device(xs: np.ndarray, loop_n: int = 1, **kw):
    nc = _get_nc(loop_n, **{**DEFAULT_KW, **kw})
    in_maps = [{"x": np.ascontiguousarray(xs[i])} for i in range(N_CORES)]
    results = None
    for attempt in range(3):
        try:
            results = run_bass_kernel_spmd(nc, in_maps, list(range(N_CORES))).results
            break
        except Exception:
            if attempt == 2:
                raise
            time.sleep(5)
            try:
                import jax
                import jax.extend.backend as _jax_backend

                jax.clear_caches()
                _jax_backend.clear_backends()
            except Exception:
                pass
    return [r["prod"] for r in results]


def _softplus64(x):
    return np.logaddexp(0.0, x)


def _reconstruct(prods, xw, n1):
    """prods: per-core [P, ROWS*GROW] bf16 slab-group sigmoid products.
    Returns per-(core, class) partial sums of softplus / t*softplus."""
    i_idx = np.arange(N_CORES, dtype=np.int64)[:, None]
    # p1[i, c] = #(t=1) dealt to core i = #{j < n1[c] : j % 8 == i}
    p1 = np.maximum(n1[None, :] - i_idx + 7, 0) // 8          # [8, C]

    g1 = TREES[0][2] // TREES[0][3]      # groups in the t1 slab (first set)
    dev_all = np.zeros((N_CORES, C), dtype=np.float64)
    dev_t1 = np.zeros((N_CORES, C), dtype=np.float64)
    for i, o in enumerate(prods):
        v = np.asarray(o).astype(np.float32).astype(np.float64)
        v = v.reshape(P, ROWS, GROW)                # (p, b, g)
        lg = np.log(v)
        # class c = b*128 + p
        t1_sum = -lg[:, :, :g1].sum(axis=2)         # (p, b)
        t0_sum = -lg[:, :, g1:].sum(axis=2)
        dev_t1[i] = t1_sum.T.reshape(C)
        dev_all[i] = dev_t1[i] + t0_sum.T.reshape(C)

    # host window: softplus of xw in f64; prefix sums for the t-boundary
    spw = _softplus64(xw.astype(np.float64))            # [8, C, WHOST]
    win_all = spw.sum(axis=2)                           # [8, C]
    cums = np.concatenate(
        [np.zeros((N_CORES, C, 1)), np.cumsum(spw, axis=2)], axis=2
    )
    wlen = np.clip(p1 - T1, 0, WHOST)
    win_t1 = np.take_along_axis(cums, wlen[:, :, None], axis=2)[:, :, 0]

    part_all = dev_all + win_all
    part_t1 = dev_t1 + win_t1

    bad = (p1 < T1) | (p1 > T1 + WHOST)
    return part_all, part_t1, bad, p1


def _finalize(s_all, s1, pos_sum, pos_prop) -> np.ndarray:
    bal = pos_prop.astype(np.float64) * B
    maj1 = pos_sum >= bal
    n_maj = np.where(maj1, pos_sum, B - pos_sum)
    n_min = B - n_maj
    s_maj = np.where(maj1, s1, s_all - s1)
    s_min = s_all - s_maj
    w_maj = bal / np.maximum(n_maj, 1.0)
    w_min = np.where(n_min > 0, (B - bal) / np.maximum(n_min, 1.0), 1.0)
    loss = (np.where(s_maj == 0, 0.0, w_maj * s_maj) + w_min * s_min).sum() / (B * C)
    return np.asarray(loss, dtype=np.float32)


def kernel(pred: np.ndarray, target: np.ndarray, pos_prop: np.ndarray) -> np.ndarray:
    pred = np.asarray(pred, dtype=np.float32)
    target = np.asarray(target, dtype=np.float32)
    pos_prop = np.asarray(pos_prop, dtype=np.float32)
    pos_sum = target.astype(np.float64).sum(axis=0)

    xs, xw, n1 = _prep_inputs(pred, target)
    prods = run_device(xs)
    part_all, part_t1, bad, p1 = _reconstruct(prods, xw, n1)

    if bad.any():
        # exact host recompute from the same fp8 values the device saw
        for i, c in zip(*np.nonzero(bad)):
            sp_dev = _softplus64(xs[i, c].astype(np.float64))
            spw = _softplus64(xw[i, c].astype(np.float64))
            part_all[i, c] = sp_dev.sum() + spw.sum()
            k = int(p1[i, c])
            if k <= T1:
                part_t1[i, c] = sp_dev[:k].sum()
            elif k <= T1 + WHOST:
                part_t1[i, c] = sp_dev[:T1].sum() + spw[: k - T1].sum()
            else:
                part_t1[i, c] = sp_dev[: k - WHOST].sum() + spw.sum()

    s_all = part_all.sum(axis=0)
    s1 = part_t1.sum(axis=0)
    return _finalize(s_all, s1, pos_sum, pos_prop)


# ---------------- benchmarking -----------------------------------------------


def bench_spmd(pred: np.ndarray, target: np.ndarray, loop_small: int = 101,
               loop_big: int = 1101, reps: int = 5, **kw):
    """Per-iteration device time via For_i loop differencing through the
    run_bass_kernel_spmd path (all 8 cores concurrently)."""
    xs, _, _ = _prep_inputs(
        np.asarray(pred, dtype=np.float32), np.asarray(target, dtype=np.float32)
    )
    ts, tb = [], []
    run_device(xs, loop_n=loop_small, **kw)  # compile+warm
    run_device(xs, loop_n=loop_big, **kw)
    for _ in range(reps):
        t0 = time.perf_counter()
        run_device(xs, loop_n=loop_small, **kw)
        ts.append(time.perf_counter() - t0)
        t0 = time.perf_counter()
        run_device(xs, loop_n=loop_big, **kw)
        tb.append(time.perf_counter() - t0)
    ns = (min(tb) - min(ts)) / (loop_big - loop_small) * 1e9
    return ns, min(ts), min(tb)


if __name__ == "__main__":
    rng = np.random.default_rng(0)
    pred = rng.standard_normal((B, C), dtype=np.float32)
    target = (rng.random((B, C)) < 0.3).astype(np.float32)
    pos_prop = np.full((C,), 0.5, dtype=np.float32)
    print(kernel(pred, target, pos_prop))


# revision 52
# speedup vs baseline: 1.0892x; 1.0227x over previous
"""Balanced BCE loss kernel for Trainium2 (8 NeuronCores, SPMD).

Math: the loss needs, per class c, the sums
    S_all[c] = sum_b softplus(x),  S1[c] = sum_b t * softplus(x)
with x = (1-2t)*pred and softplus(x) = -ln sigmoid(-x). Both sums are
order-invariant over the batch, so the HOST counting-sorts each class's
65536 elements t-first and deals them round-robin to the 8 cores. Per
(core, class) row of 8192 sorted columns:
  - cols [0, 1792)      : guaranteed pure t=1  -> device slab 0
  - cols [1792, 2816)   : boundary window      -> HOST (1024 cols, f64;
                          the t=1/t=0 boundary p1 = #(t=1)/8 ~ 2457 +- 15
                          for Bernoulli(0.3) targets: +-20 sigma margins)
  - cols [2816, 8192)   : guaranteed pure t=0  -> device slabs 1-4
Device rows are 7168 fp8 columns in slabs of (1792 | 1792,1792,1280,512raw),
transposed [class, col]: one [128, 4, 7168] block per iteration (partition p
holds classes {p, 128+p, 256+p, 384+p}). The ACT sigmoid chain
s = Sigmoid(-x) (fp8 -> bf16, 1 elem/lane/cycle @1.2GHz, 23.9us/core of
elements + 5 x ~185ns instruction overheads) is the critical path;
everything else is scheduled to hide inside it:
  - the first 1792 cols live in a single-buffered prefetch tile that
    iteration i-1 reloads for iteration i, so the first sigmoid issues at
    the top of the loop body instead of waiting out the post-barrier DMA
    latency (~2.4us); the remaining x streams on SWDGE in slab-sized chunks
  - per-slab DVE halving trees collapse each t-pure slab into groups:
    ln prod = sum ln s, so the host recovers exact per-group log-sigmoid
    sums with one np.log per product (group purity holds: each group lives
    inside one t-pure slab of one class); tapered slab widths let the tree
    of slab k hide under the ACT of slab k+1
  - the three 1792-slabs use k=8 trees; the late 1280-slab uses a shallow
    k=2 tree and the last 512-col ACT piece writes raw bf16 sigmoids
    straight into a persistent tail tile, so no DVE work remains after the
    final sigmoid
  - products ship as one bulk DMA (first 3 tree sets, issued mid-chain)
    plus ONE deferred tail DMA issued at the start of the NEXT iteration -
    its issue and ~1.7us queue drain hide under the ACT chain instead of
    extending the end-of-iteration barrier (single-shot NEFFs ship the
    tail at the end instead). 1.8 MiB/core out vs 8 MiB/core for the
    previous pair-product kernel; total HBM traffic 5.3 MiB/core vs 12.
Host finalize: group logs + window softplus + exact integer bookkeeping of
the t=1 prefix per (core, class); any row whose boundary leaves the window
(impossible for the reference distribution) is recomputed exactly on host
from the same fp8 values the device saw.

Measured (loop-differenced through run_bass_kernel_spmd, 8 cores, same day
and method): ~29-31us/iter vs 53.4us/iter for the previous pair-product
kernel under +-3us tunnel noise. CoreSim models 25.5us/iter steady (ACT
chain 24.8us + ~0.7us barrier) vs 35.0us for the old kernel; the larger
real-path gain comes from shedding the old kernel's HBM contention.
"""

import sys
import time
from contextlib import ExitStack

import numpy as np
import ml_dtypes

sys.path.insert(0, "/opt/trn_rl_repo")

from concourse import bacc, mybir, tile  # noqa: E402
from concourse.bass_utils import run_bass_kernel_spmd  # noqa: E402

B, C = 65536, 512
N_CORES = 8
P = 128
ROWS = C // P            # 4 class rows of 128 per partition, one block
COLS = B // N_CORES      # 8192 sorted columns per (core, class)
T1 = 1792                # t=1-pure device cols (slab 0)
WHOST = 1024             # host window cols, sorted order [T1, T1+WHOST)
DCOLS = COLS - WHOST     # 7168 device cols per class row
T0 = DCOLS - T1          # 5376 t=0-pure device cols
K_GROUP = 8              # sigmoids per product group (3 halving levels)

# tree sets: (col offset, n equal slabs, slab width, group size); first set
# is the t1 slab; the late 1280-slab uses a shallow k=2 tree so its single
# DVE level hides under the raw-tail ACT piece
TREES = ((0, 1, 1792, 8), (1792, 1, 1792, 8), (3584, 1, 1792, 8),
         (5376, 1, 1280, 2))
RAW_OFF = 6656           # last 512 cols ship as raw bf16 sigmoids: their
RAW_W = DCOLS - RAW_OFF  # ACT piece writes straight into the tail tile,
                         # so no DVE work remains after the final sigmoid
# ACT piece column boundaries (the tree of piece k hides under the ACT of
# piece k+1; the [0:1792) head is prefetched so its data is ready at body
# start)
ACT_BOUNDS = (0, 1792, 3584, 5376, 6656, 7168)
G_PROD = sum(ns * w // k for _, ns, w, k in TREES)       # 1312 products per row
GROW = G_PROD + RAW_W                                    # 1824 out cols per row
BULK = sum(ns * w // k for _, ns, w, k in TREES[:3])     # 672: early-out sets
TAILW = GROW - BULK                                      # 1152: deferred tail

F32 = mybir.dt.float32
BF16 = mybir.dt.bfloat16
FP8 = mybir.dt.float8e4

FP8_NP = ml_dtypes.float8_e4m3
BF16_NP = ml_dtypes.bfloat16

_CACHE = {}


def _build(loop_n: int = 1, w_bufs: int = 2, mode: str = "full",
           dma_bounds: tuple = (0, 1792, 3584, 5376, 7168),
           prefetch: int = 1792):
    nc = bacc.Bacc(
        "TRN2", target_bir_lowering=False, debug=False, num_devices=N_CORES
    )
    x = nc.dram_tensor("x", [C, DCOLS], FP8, kind="ExternalInput").ap()
    prod = nc.dram_tensor(
        "prod", [P, ROWS * GROW], BF16, kind="ExternalOutput"
    ).ap()
    prod_v = prod.rearrange("p (b g) -> p b g", b=ROWS)
    # partition p holds classes {p, 128+p, 256+p, 384+p} (row b = class block)
    x_v = x.rearrange("(b p) q -> p b q", p=P)

    with tile.TileContext(nc) as tc, ExitStack() as stack:
        io = stack.enter_context(tc.tile_pool(name="io", bufs=1))
        sp = stack.enter_context(tc.tile_pool(name="sp", bufs=1))
        wk = stack.enter_context(tc.tile_pool(name="wk", bufs=w_bufs))
        pf = stack.enter_context(tc.tile_pool(name="pf", bufs=1))
        # hoist the sigmoid ACT_TABLE_LOAD out of the loop body (the DMA of
        # the result keeps the warm activation from being dead-code removed)
        warm = wk.tile([P, 2], BF16, tag="warm")
        nc.vector.memset(warm[:], 0.0)
        nc.scalar.activation(
            warm[:], warm[:], mybir.ActivationFunctionType.Sigmoid, scale=-1.0
        )
        nc.sync.dma_start(out=prod[:, 0:2], in_=warm[:])
        # software pipeline: the first `prefetch` columns live in a
        # single-buffered tile that iteration i-1 loads for iteration i, so
        # the first sigmoid starts right at the top of the body instead of
        # waiting out the post-barrier DMA latency
        if prefetch:
            x_pf = pf.tile([P, ROWS, prefetch], FP8, tag="xpf")
            nc.gpsimd.dma_start(out=x_pf[:], in_=x_v[:, :, 0:prefetch])
        # tail products live in a single-buffered tile whose DMA-out is
        # deferred to the START of the next iteration, so its issue and
        # queue-drain hide under the ACT chain instead of extending the
        # end-of-iteration barrier (iteration 0 ships the memset content)
        tail_t = pf.tile([P, ROWS, TAILW], BF16, tag="tail")
        nc.vector.memset(tail_t[:], 1.0)
        if loop_n > 1:
            stack.enter_context(tc.For_i(0, loop_n, 1))
            if mode != "act":
                nc.sync.dma_start(
                    out=prod_v[:, :, BULK:GROW], in_=tail_t[:]
                )

        n_tree_sets = len(TREES)
        x_t = io.tile([P, ROWS, DCOLS], FP8, tag="x")
        for a, b in zip(dma_bounds[:-1], dma_bounds[1:]):
            if b <= prefetch:
                continue  # covered by the prefetch tile
            a = max(a, prefetch)
            nc.gpsimd.dma_start(out=x_t[:, :, a:b], in_=x_v[:, :, a:b])
        s_t = sp.tile([P, ROWS, DCOLS], BF16, tag="s")
        gather = wk.tile([P, ROWS, BULK], BF16, tag="gather")
        done = 0
        ti = 0  # next tree set awaiting activation coverage
        goff = 0
        for a, b in zip(ACT_BOUNDS[:-1], ACT_BOUNDS[1:]):
            if b <= prefetch:
                src_x = x_pf[:, :, a:b]
            else:
                src_x = x_t[:, :, a:b]
            if a >= RAW_OFF:
                # raw-sigmoid tail goes straight into the tail tile
                o0 = G_PROD - BULK + a - RAW_OFF
                dst = tail_t[:, :, o0:o0 + b - a]
            else:
                dst = s_t[:, :, a:b]
            nc.scalar.activation(
                dst, src_x,
                mybir.ActivationFunctionType.Sigmoid, scale=-1.0,
            )
            done = b
            if prefetch and done == prefetch:
                # refill the prefetch tile for the next iteration now
                # that its last reader has been issued
                nc.gpsimd.dma_start(
                    out=x_pf[:], in_=x_v[:, :, 0:prefetch]
                )
            if mode == "act":
                continue
            # emit trees whose slabs are fully activated; final levels write
            # into the contiguous gather / tail tiles
            while ti < n_tree_sets:
                off, nsl, w, kg = TREES[ti]
                if off + nsl * w > done:
                    break
                src = s_t[:, :, off:off + nsl * w].rearrange(
                    "p b (v q) -> p b v q", v=nsl
                )
                cur = w
                g = w // kg
                ng = nsl * g
                while cur > g:
                    half = cur // 2
                    if half == g and goff + ng <= BULK:
                        nxt = gather[:, :, goff:goff + ng].rearrange(
                            "p b (v q) -> p b v q", v=nsl
                        )
                    elif half == g:
                        to = goff - BULK
                        nxt = tail_t[:, :, to:to + ng].rearrange(
                            "p b (v q) -> p b v q", v=nsl
                        )
                    else:
                        nxt = wk.tile(
                            [P, ROWS, nsl, half], BF16, tag=f"t{off}_{half}"
                        )
                    nc.vector.tensor_mul(
                        nxt[:], src[:, :, :, 0:half], src[:, :, :, half:cur]
                    )
                    src, cur = nxt, half
                goff += ng
                ti += 1
                # ship the bulk of the products as soon as the first
                # three tree sets land; the rest rides the deferred tail
                if mode == "full" and goff == BULK:
                    nc.sync.dma_start(
                        out=prod_v[:, :, 0:BULK],
                        in_=gather[:, :, 0:BULK],
                    )
        if mode == "full" and loop_n == 1:
            # single-shot: no next iteration to ship the tail
            nc.sync.dma_start(
                out=prod_v[:, :, BULK:GROW], in_=tail_t[:]
            )
        if mode != "full":
            dummy = wk.tile([P, 2], BF16, tag="dummy")
            nc.vector.memset(dummy[:], 1.0)
            nc.sync.dma_start(out=prod[:, 0:2], in_=dummy[:])

    nc.compile()
    return nc


def _get_nc(loop_n: int = 1, **kw):
    key = (loop_n, tuple(sorted(kw.items())))
    if key not in _CACHE:
        _CACHE[key] = _build(loop_n, **kw)
    return _CACHE[key]


DEFAULT_KW = dict()


def _prep_inputs(pred: np.ndarray, target: np.ndarray):
    """Host-side: per-class counting sort by t, deal to cores, split into
    device (t-pure) columns and host (boundary window) columns.

    Returns (xs, xw, n1):
      xs: [N_CORES, C, DCOLS] fp8 device inputs (slab0 t1 | slabs1-4 t0)
      xw: [N_CORES, C, WHOST] f32 host window columns
      n1: [C] int64 per-class t=1 counts
    """
    t = target != 0.0
    n1 = t.sum(axis=0, dtype=np.int64)
    x = ((1.0 - 2.0 * target) * pred).astype(FP8_NP)

    # global sorted position per element: t=1 elements first (stable order);
    # rank among t=0 = row_index - (#t=1 so far), so one cumsum suffices
    c1 = np.cumsum(t, axis=0, dtype=np.int32)
    rows = np.arange(B, dtype=np.int32)[:, None]
    pos = np.where(t, c1 - 1, n1.astype(np.int32)[None, :] + rows - c1)
    # deal to cores round-robin; scatter everything into a staging array
    # [core, class, col] and slice device/window parts out afterwards
    cc = np.broadcast_to(np.arange(C, dtype=np.int32)[None, :], pos.shape)
    lin = ((pos & 7) * C + cc) * COLS + (pos >> 3)
    stage = np.empty(N_CORES * C * COLS, dtype=np.uint8)
    stage[lin.reshape(-1)] = x.view(np.uint8).reshape(-1)
    stage = stage.reshape(N_CORES, C, COLS)

    xs = np.empty((N_CORES, C, DCOLS), dtype=np.uint8)
    xs[:, :, :T1] = stage[:, :, :T1]
    xs[:, :, T1:] = stage[:, :, T1 + WHOST:]
    xw = stage[:, :, T1:T1 + WHOST]

    xs = xs.view(FP8_NP)
    xw_f = xw.view(FP8_NP).astype(np.float32)
    return xs, xw_f, n1


def run_device(xs: np.ndarray, loop_n: int = 1, **kw):
    nc = _get_nc(loop_n, **{**DEFAULT_KW, **kw})
    in_maps = [{"x": np.ascontiguousarray(xs[i])} for i in range(N_CORES)]
    results = None
    for attempt in range(3):
        try:
            results = run_bass_kernel_spmd(nc, in_maps, list(range(N_CORES))).results
            break
        except Exception:
            if attempt == 2:
                raise
            time.sleep(5)
            try:
                import jax
                import jax.extend.backend as _jax_backend

                jax.clear_caches()
                _jax_backend.clear_backends()
            except Exception:
                pass
    return [r["prod"] for r in results]


def _softplus64(x):
    return np.logaddexp(0.0, x)


def _reconstruct(prods, xw, n1):
    """prods: per-core [P, ROWS*GROW] bf16 slab-group sigmoid products.
    Returns per-(core, class) partial sums of softplus / t*softplus."""
    i_idx = np.arange(N_CORES, dtype=np.int64)[:, None]
    # p1[i, c] = #(t=1) dealt to core i = #{j < n1[c] : j % 8 == i}
    p1 = np.maximum(n1[None, :] - i_idx + 7, 0) // 8          # [8, C]

    g1 = TREES[0][2] // TREES[0][3]      # groups in the t1 slab (first set)
    dev_all = np.zeros((N_CORES, C), dtype=np.float64)
    dev_t1 = np.zeros((N_CORES, C), dtype=np.float64)
    for i, o in enumerate(prods):
        v = np.asarray(o).astype(np.float32).astype(np.float64)
        v = v.reshape(P, ROWS, GROW)                # (p, b, g)
        lg = np.log(v)
        # class c = b*128 + p
        t1_sum = -lg[:, :, :g1].sum(axis=2)         # (p, b)
        t0_sum = -lg[:, :, g1:].sum(axis=2)
        dev_t1[i] = t1_sum.T.reshape(C)
        dev_all[i] = dev_t1[i] + t0_sum.T.reshape(C)

    # host window: softplus of xw in f64; prefix sums for the t-boundary
    spw = _softplus64(xw.astype(np.float64))            # [8, C, WHOST]
    win_all = spw.sum(axis=2)                           # [8, C]
    cums = np.concatenate(
        [np.zeros((N_CORES, C, 1)), np.cumsum(spw, axis=2)], axis=2
    )
    wlen = np.clip(p1 - T1, 0, WHOST)
    win_t1 = np.take_along_axis(cums, wlen[:, :, None], axis=2)[:, :, 0]

    part_all = dev_all + win_all
    part_t1 = dev_t1 + win_t1

    bad = (p1 < T1) | (p1 > T1 + WHOST)
    return part_all, part_t1, bad, p1


def _finalize(s_all, s1, pos_sum, pos_prop) -> np.ndarray:
    bal = pos_prop.astype(np.float64) * B
    maj1 = pos_sum >= bal
    n_maj = np.where(maj1, pos_sum, B - pos_sum)
    n_min = B - n_maj
    s_maj = np.where(maj1, s1, s_all - s1)
    s_min = s_all - s_maj
    w_maj = bal / np.maximum(n_maj, 1.0)
    w_min = np.where(n_min > 0, (B - bal) / np.maximum(n_min, 1.0), 1.0)
    loss = (np.where(s_maj == 0, 0.0, w_maj * s_maj) + w_min * s_min).sum() / (B * C)
    return np.asarray(loss, dtype=np.float32)


def kernel(pred: np.ndarray, target: np.ndarray, pos_prop: np.ndarray) -> np.ndarray:
    pred = np.asarray(pred, dtype=np.float32)
    target = np.asarray(target, dtype=np.float32)
    pos_prop = np.asarray(pos_prop, dtype=np.float32)
    pos_sum = target.astype(np.float64).sum(axis=0)

    xs, xw, n1 = _prep_inputs(pred, target)
    prods = run_device(xs)
    part_all, part_t1, bad, p1 = _reconstruct(prods, xw, n1)

    if bad.any():
        # exact host recompute from the same fp8 values the device saw
        for i, c in zip(*np.nonzero(bad)):
            sp_dev = _softplus64(xs[i, c].astype(np.float64))
            spw = _softplus64(xw[i, c].astype(np.float64))
            part_all[i, c] = sp_dev.sum() + spw.sum()
            k = int(p1[i, c])
            if k <= T1:
                part_t1[i, c] = sp_dev[:k].sum()
            elif k <= T1 + WHOST:
                part_t1[i, c] = sp_dev[:T1].sum() + spw[: k - T1].sum()
            else:
                part_t1[i, c] = sp_dev[: k - WHOST].sum() + spw.sum()

    s_all = part_all.sum(axis=0)
    s1 = part_t1.sum(axis=0)
    return _finalize(s_all, s1, pos_sum, pos_prop)


# ---------------- benchmarking -----------------------------------------------


def bench_spmd(pred: np.ndarray, target: np.ndarray, loop_small: int = 101,
               loop_big: int = 1101, reps: int = 5, **kw):
    """Per-iteration device time via For_i loop differencing through the
    run_bass_kernel_spmd path (all 8 cores concurrently)."""
    xs, _, _ = _prep_inputs(
        np.asarray(pred, dtype=np.float32), np.asarray(target, dtype=np.float32)
    )
    ts, tb = [], []
    run_device(xs, loop_n=loop_small, **kw)  # compile+warm
    run_device(xs, loop_n=loop_big, **kw)
    for _ in range(reps):
        t0 = time.perf_counter()
        run_device(xs, loop_n=loop_small, **kw)
        ts.append(time.perf_counter() - t0)
        t0 = time.perf_counter()
        run_device(xs, loop_n=loop_big, **kw)
        tb.append(time.perf_counter() - t0)
    ns = (min(tb) - min(ts)) / (loop_big - loop_small) * 1e9
    return ns, min(ts), min(tb)


if __name__ == "__main__":
    rng = np.random.default_rng(0)
    pred = rng.standard_normal((B, C), dtype=np.float32)
    target = (rng.random((B, C)) < 0.3).astype(np.float32)
    pos_prop = np.full((C,), 0.5, dtype=np.float32)
    print(kernel(pred, target, pos_prop))
